# revision 20
# baseline (speedup 1.0000x reference)
"""MinGRU layer kernel for 8 Trainium2 NeuronCores.

Problem: x (4, 8192, 1024) f32; Wz, Wh (1024, 1024); bz, bh (1024,)
    z = sigmoid(x @ Wz + bz); h_tilde = x @ Wh + bh
    h_t = (1 - z_t) * h_{t-1} + z_t * h_tilde_t   (scan over seq, h_{-1} = 0)

Sharding: 8 cores = 4 batches x 2 output-dim halves. The scan is
independent per (batch, dim), so each core owns a full-sequence scan for
one batch and 512 of the 1024 output dims -- no cross-core traffic.

Layout: host pre-transposes x to (d_in, seq) fp16 per batch. On device the
matmul keeps W stationary (lhsT = W tile, natural layout) and streams x^T,
producing (d_out, seq) tiles in PSUM -- exactly the layout
tensor_tensor_scan needs (scan runs along the free/seq axis, one recurrence
per partition/dim). ScalarE computes a = sigmoid(-(z_pre)) and
z = sigmoid(z_pre) straight out of PSUM; VectorE fuses b = (h_pre + bh) * z
and then runs the scan. Output h^T (512, 8192) f32 is written contiguously;
the host transposes back during the gather.
"""

import sys

if "/opt/trn_rl_repo" not in sys.path:
    sys.path.insert(0, "/opt/trn_rl_repo")

import numpy as np

from concourse import bass, mybir
from concourse.tile import TileContext
from concourse.bass_utils import run_bass_kernel_spmd

BATCH, SEQ, D = 4, 8192, 1024
DH = 512            # output dims per core
N_CORES = 8
CHUNK = 1024        # seq elements per x^T DMA chunk
NCHUNK = SEQ // CHUNK
NS5 = CHUNK // 512  # 512-wide matmul sub-chunks per chunk
NM = DH // 128      # output-dim tiles per core
NK = D // 128       # contraction tiles

F16 = mybir.dt.float16
F32 = mybir.dt.float32
AF = mybir.ActivationFunctionType
OP = mybir.AluOpType


_WAIT_LIMIT = 1  # this walrus build rejects multiple sem waits per instruction


def _split_sync_waits(nc):
    """Move excess semaphore waits (beyond _WAIT_LIMIT) off each instruction
    onto same-engine nops inserted immediately before it. Waits only gate
    execution, so hoisting some onto a preceding nop in the same engine
    stream is semantics-preserving."""
    import bass_rust

    n_extra = 0
    for fn in nc.m.functions:
        for blk in fn.blocks:
            insts = blk.instructions
            out = []
            for inst in insts:
                si = inst.sync_info
                if si is not None and si.on_wait and len(si.on_wait) > _WAIT_LIMIT:
                    waits = list(si.on_wait)
                    head, tail = waits[:-_WAIT_LIMIT], waits[-_WAIT_LIMIT:]
                    for j in range(0, len(head), _WAIT_LIMIT):
                        n_extra += 1
                        nop = bass_rust.InstNoOp(
                            name=f"{inst.name}-waitsplit{j}",
                            engine=inst.engine,
                            sync_info=type(si)(
                                on_wait=head[j:j + _WAIT_LIMIT], on_update=[]
                            ),
                            bass_nofuse=True,
                        )
                        nc.register_instruction(nop, overwrite=True)
                        out.append(nop)
                    si.on_wait = tail
                out.append(inst)
            if n_extra:
                blk.instructions = out
    return n_extra


def _build_program():
    nc = bass.Bass("TRN2", target_bir_lowering=False, debug=False)

    xT = nc.dram_tensor("xT", [D, SEQ], F16, kind="ExternalInput").ap()
    wz = nc.dram_tensor("wz", [D, DH], F16, kind="ExternalInput").ap()
    wh = nc.dram_tensor("wh", [D, DH], F16, kind="ExternalInput").ap()
    # biases packed: [bz | bzn | bh] x NM m-tiles -> (128, 3*NM), one DMA
    bias = nc.dram_tensor("bias", [128, 3 * NM], F32, kind="ExternalInput").ap()
    hT = nc.dram_tensor("hT", [DH, SEQ], F32, kind="ExternalOutput").ap()

    with TileContext(nc) as tc:
        with (
            tc.tile_pool(name="weights", bufs=1) as wpool,
            tc.tile_pool(name="bias", bufs=1) as biaspool,
            tc.tile_pool(name="xt", bufs=3) as xpool,
            tc.tile_pool(name="a", bufs=3) as apool,
            tc.tile_pool(name="z", bufs=3) as zpool,
            tc.tile_pool(name="b", bufs=3) as bpool,
            tc.tile_pool(name="h", bufs=3) as hpool,
            tc.tile_pool(name="psz", bufs=4, space="PSUM") as pszpool,
            tc.tile_pool(name="psh", bufs=4, space="PSUM") as pshpool,
        ):
            # Weights resident for the whole kernel: (128 k, 512 m) per
            # k-tile. Weights/bias/output DMAs ride the scalar HWDGE ring so
            # the sync ring is dedicated to x^T prefetch.
            bias_t = biaspool.tile([128, 3 * NM], F32, tag="bias")
            nc.scalar.dma_start(out=bias_t[:], in_=bias[:])
            bz_t = [bias_t[:, m:m + 1] for m in range(NM)]
            bzn_t = [bias_t[:, NM + m:NM + m + 1] for m in range(NM)]
            bh_t = [bias_t[:, 2 * NM + m:2 * NM + m + 1] for m in range(NM)]

            # Per-(kt, m) weight tiles, loaded m-major so the first matmul
            # group's weights land first and the PE can start early.
            wz_t = [[None] * NM for _ in range(NK)]
            wh_t = [[None] * NM for _ in range(NK)]
            for m in range(NM):
                for w_t, src, nm in ((wz_t, wz, "wz"), (wh_t, wh, "wh")):
                    for kt in range(NK):
                        w1 = wpool.tile([128, 128], F16, tag=f"{nm}{kt}_{m}")
                        nc.scalar.dma_start(
                            out=w1[:],
                            in_=src[kt * 128:(kt + 1) * 128,
                                    m * 128:(m + 1) * 128],
                        )
                        w_t[kt][m] = w1

            last_h = [None] * NM
            for c in range(NCHUNK):
                xt = []
                for kt in range(NK):
                    t = xpool.tile([128, CHUNK], F16, tag=f"x{kt}")
                    nc.sync.dma_start(
                        out=t[:],
                        in_=xT[kt * 128:(kt + 1) * 128,
                               c * CHUNK:(c + 1) * CHUNK],
                    )
                    xt.append(t)

                h_big = []
                for m in range(NM):
                    h_m = hpool.tile([128, CHUNK], F32, tag=f"h{m}")
                    h_big.append(h_m)
                for s5 in range(NS5):
                    for m in range(NM):
                        psz = pszpool.tile([128, 512], F32)
                        psh = pshpool.tile([128, 512], F32)
                        for kt in range(NK):
                            nc.tensor.matmul(
                                psz[:],
                                wz_t[kt][m][:],
                                xt[kt][:, s5 * 512:(s5 + 1) * 512],
                                start=(kt == 0),
                                stop=(kt == NK - 1),
                            )
                        for kt in range(NK):
                            nc.tensor.matmul(
                                psh[:],
                                wh_t[kt][m][:],
                                xt[kt][:, s5 * 512:(s5 + 1) * 512],
                                start=(kt == 0),
                                stop=(kt == NK - 1),
                            )
                        # a = 1 - sigmoid(z_pre + bz) = sigmoid(-z_pre - bz)
                        a_t = apool.tile([128, 512], F32)
                        nc.scalar.activation(a_t[:], psz[:], AF.Sigmoid,
                                             bias=bzn_t[m][:], scale=-1.0)
                        z_t = zpool.tile([128, 512], F32)
                        nc.scalar.activation(z_t[:], psz[:], AF.Sigmoid,
                                             bias=bz_t[m][:], scale=1.0)
                        # b = (h_pre + bh) * z
                        b_t = bpool.tile([128, 512], F32)
                        nc.vector.scalar_tensor_tensor(
                            b_t[:], psh[:], bh_t[m][:], z_t[:],
                            op0=OP.add, op1=OP.mult,
                        )
                        # h_t = a_t * h_{t-1} + b_t along seq
                        h_t = h_big[m][:, s5 * 512:(s5 + 1) * 512]
                        init = 0.0 if last_h[m] is None else last_h[m][:, -1:]
                        nc.vector.tensor_tensor_scan(
                            h_t, a_t[:], b_t[:], init,
                            op0=OP.mult, op1=OP.add,
                        )
                        last_h[m] = h_t
                for mm in range(NM):
                    nc.sync.dma_start(
                        out=hT[mm * 128:(mm + 1) * 128,
                               c * CHUNK:(c + 1) * CHUNK],
                        in_=h_big[mm][:],
                    )
    _split_sync_waits(nc)
    return nc


_NC_CACHE = None


def _get_program():
    global _NC_CACHE
    if _NC_CACHE is None:
        _NC_CACHE = _build_program()
    return _NC_CACHE


def _make_in_maps(x, Wz, bz, Wh, bh):
    xT16 = [np.ascontiguousarray(x[b].T).astype(np.float16) for b in range(BATCH)]
    wzh = [np.ascontiguousarray(Wz[:, c * DH:(c + 1) * DH]).astype(np.float16)
           for c in range(2)]
    whh = [np.ascontiguousarray(Wh[:, c * DH:(c + 1) * DH]).astype(np.float16)
           for c in range(2)]
    # bias[p, m] = bz[m*128+p]; columns [0:NM]=bz, [NM:2NM]=-bz, [2NM:3NM]=bh
    biases = []
    for c in range(2):
        bzc = bz[c * DH:(c + 1) * DH].astype(np.float32).reshape(NM, 128).T
        bhc = bh[c * DH:(c + 1) * DH].astype(np.float32).reshape(NM, 128).T
        biases.append(np.ascontiguousarray(np.hstack([bzc, -bzc, bhc])))
    in_maps = []
    for i in range(N_CORES):
        b, c = i // 2, i % 2
        in_maps.append({
            "xT": xT16[b], "wz": wzh[c], "wh": whh[c], "bias": biases[c],
        })
    return in_maps


def _run(x, Wz, bz, Wh, bh, trace=False, trace_cores=None):
    nc = _get_program()
    in_maps = _make_in_maps(x, Wz, bz, Wh, bh)
    res = run_bass_kernel_spmd(
        nc, in_maps, list(range(N_CORES)), trace=trace, trace_cores=trace_cores
    )
    out = np.empty((BATCH, SEQ, D), dtype=np.float32)
    for i in range(N_CORES):
        b, c = i // 2, i % 2
        out[b, :, c * DH:(c + 1) * DH] = res.results[i]["hT"].T
    return out, res


def kernel(x, Wz, bz, Wh, bh):
    x = np.asarray(x, dtype=np.float32)
    Wz = np.asarray(Wz, dtype=np.float32)
    Wh = np.asarray(Wh, dtype=np.float32)
    bz = np.asarray(bz, dtype=np.float32)
    bh = np.asarray(bh, dtype=np.float32)
    out, _ = _run(x, Wz, bz, Wh, bh, trace=False)
    return out


# revision 23
# speedup vs baseline: 1.1213x; 1.1213x over previous
"""MinGRU layer kernel for 8 Trainium2 NeuronCores.

Problem: x (4, 8192, 1024) f32; Wz, Wh (1024, 1024); bz, bh (1024,)
    z = sigmoid(x @ Wz + bz); h_tilde = x @ Wh + bh
    h_t = (1 - z_t) * h_{t-1} + z_t * h_tilde_t   (scan over seq, h_{-1} = 0)

Sharding: 8 cores = 4 batches x 2 output-dim halves. The scan is
independent per (batch, dim), so each core owns a full-sequence scan for
one batch and 512 of the 1024 output dims -- no cross-core traffic.

Layout: host pre-transposes x to (d_in, seq) fp16 per batch. On device the
matmul keeps W stationary (lhsT = W tile, natural layout) and streams x^T,
producing (d_out, seq) tiles in PSUM -- exactly the layout
tensor_tensor_scan needs (scan runs along the free/seq axis, one recurrence
per partition/dim). ScalarE computes a = sigmoid(-(z_pre)) and
z = sigmoid(z_pre) straight out of PSUM; VectorE fuses b = (h_pre + bh) * z
and then runs the scan. Output h^T (512, 8192) f32 is written contiguously;
the host transposes back during the gather.
"""

import sys

if "/opt/trn_rl_repo" not in sys.path:
    sys.path.insert(0, "/opt/trn_rl_repo")

import numpy as np

from concourse import bass, mybir
from concourse.tile import TileContext
from concourse.bass_utils import run_bass_kernel_spmd

BATCH, SEQ, D = 4, 8192, 1024
DH = 512            # output dims per core
N_CORES = 8
CHUNK = 1024        # seq elements per x^T DMA chunk
NCHUNK = SEQ // CHUNK
NS5 = CHUNK // 512  # 512-wide matmul sub-chunks per chunk
NM = DH // 128      # output-dim tiles per core
NK = D // 128       # contraction tiles

F16 = mybir.dt.float16
F32 = mybir.dt.float32
AF = mybir.ActivationFunctionType
OP = mybir.AluOpType


_WAIT_LIMIT = 1  # this walrus build rejects multiple sem waits per instruction


def _split_sync_waits(nc):
    """Move excess semaphore waits (beyond _WAIT_LIMIT) off each instruction
    onto same-engine nops inserted immediately before it. Waits only gate
    execution, so hoisting some onto a preceding nop in the same engine
    stream is semantics-preserving."""
    import bass_rust

    n_extra = 0
    for fn in nc.m.functions:
        for blk in fn.blocks:
            insts = blk.instructions
            out = []
            for inst in insts:
                si = inst.sync_info
                if si is not None and si.on_wait and len(si.on_wait) > _WAIT_LIMIT:
                    waits = list(si.on_wait)
                    head, tail = waits[:-_WAIT_LIMIT], waits[-_WAIT_LIMIT:]
                    for j in range(0, len(head), _WAIT_LIMIT):
                        n_extra += 1
                        nop = bass_rust.InstNoOp(
                            name=f"{inst.name}-waitsplit{j}",
                            engine=inst.engine,
                            sync_info=type(si)(
                                on_wait=head[j:j + _WAIT_LIMIT], on_update=[]
                            ),
                            bass_nofuse=True,
                        )
                        nc.register_instruction(nop, overwrite=True)
                        out.append(nop)
                    si.on_wait = tail
                out.append(inst)
            if n_extra:
                blk.instructions = out
    return n_extra


def _build_program():
    nc = bass.Bass("TRN2", target_bir_lowering=False, debug=False)

    xT = nc.dram_tensor("xT", [D, SEQ], F16, kind="ExternalInput").ap()
    wz = nc.dram_tensor("wz", [D, DH], F16, kind="ExternalInput").ap()
    wh = nc.dram_tensor("wh", [D, DH], F16, kind="ExternalInput").ap()
    # biases packed: [bz | bzn | bh] x NM m-tiles -> (128, 3*NM), one DMA
    bias = nc.dram_tensor("bias", [128, 3 * NM], F32, kind="ExternalInput").ap()
    hT = nc.dram_tensor("hT", [DH, SEQ], F32, kind="ExternalOutput").ap()

    with TileContext(nc) as tc:
        with (
            tc.tile_pool(name="weights", bufs=1) as wpool,
            tc.tile_pool(name="bias", bufs=1) as biaspool,
            tc.tile_pool(name="xt", bufs=3) as xpool,
            tc.tile_pool(name="a", bufs=3) as apool,
            tc.tile_pool(name="z", bufs=3) as zpool,
            tc.tile_pool(name="b", bufs=3) as bpool,
            tc.tile_pool(name="h", bufs=3) as hpool,
            tc.tile_pool(name="psz", bufs=3, space="PSUM") as pszpool,
            tc.tile_pool(name="psh", bufs=3, space="PSUM") as pshpool,
        ):
            # Weights resident for the whole kernel: (128 k, 512 m) per
            # k-tile. Weights/bias/output DMAs ride the scalar HWDGE ring so
            # the sync ring is dedicated to x^T prefetch.
            bias_t = biaspool.tile([128, 3 * NM], F32, tag="bias")
            nc.scalar.dma_start(out=bias_t[:], in_=bias[:])
            bz_t = [bias_t[:, m:m + 1] for m in range(NM)]
            bzn_t = [bias_t[:, NM + m:NM + m + 1] for m in range(NM)]
            bh_t = [bias_t[:, 2 * NM + m:2 * NM + m + 1] for m in range(NM)]

            wz_b, wh_b = [], []
            for kt in range(NK):
                w1 = wpool.tile([128, DH], F16, tag=f"wz{kt}")
                nc.scalar.dma_start(out=w1[:], in_=wz[kt * 128:(kt + 1) * 128, :])
                wz_b.append(w1)
            for kt in range(NK):
                w2 = wpool.tile([128, DH], F16, tag=f"wh{kt}")
                nc.scalar.dma_start(out=w2[:], in_=wh[kt * 128:(kt + 1) * 128, :])
                wh_b.append(w2)
            wz_t = [[wz_b[kt][:, m * 128:(m + 1) * 128] for m in range(NM)]
                    for kt in range(NK)]
            wh_t = [[wh_b[kt][:, m * 128:(m + 1) * 128] for m in range(NM)]
                    for kt in range(NK)]

            last_h = [None] * NM
            for c in range(NCHUNK):
                xt = []
                for kt in range(NK):
                    t = xpool.tile([128, CHUNK], F16, tag=f"x{kt}")
                    nc.sync.dma_start(
                        out=t[:],
                        in_=xT[kt * 128:(kt + 1) * 128,
                               c * CHUNK:(c + 1) * CHUNK],
                    )
                    xt.append(t)

                h_big = []
                for m in range(NM):
                    h_m = hpool.tile([128, CHUNK], F32, tag=f"h{m}")
                    h_big.append(h_m)
                for s5 in range(NS5):
                    for m in range(NM):
                        psz = pszpool.tile([128, 512], F32)
                        psh = pshpool.tile([128, 512], F32)
                        for kt in range(NK):
                            nc.tensor.matmul(
                                psz[:],
                                wz_t[kt][m][:],
                                xt[kt][:, s5 * 512:(s5 + 1) * 512],
                                start=(kt == 0),
                                stop=(kt == NK - 1),
                            )
                        for kt in range(NK):
                            nc.tensor.matmul(
                                psh[:],
                                wh_t[kt][m][:],
                                xt[kt][:, s5 * 512:(s5 + 1) * 512],
                                start=(kt == 0),
                                stop=(kt == NK - 1),
                            )
                        # a = 1 - sigmoid(z_pre + bz) = sigmoid(-z_pre - bz)
                        a_t = apool.tile([128, 512], F32)
                        nc.scalar.activation(a_t[:], psz[:], AF.Sigmoid,
                                             bias=bzn_t[m][:], scale=-1.0)
                        z_t = zpool.tile([128, 512], F32)
                        nc.scalar.activation(z_t[:], psz[:], AF.Sigmoid,
                                             bias=bz_t[m][:], scale=1.0)
                        # b = (h_pre + bh) * z
                        b_t = bpool.tile([128, 512], F32)
                        nc.vector.scalar_tensor_tensor(
                            b_t[:], psh[:], bh_t[m][:], z_t[:],
                            op0=OP.add, op1=OP.mult,
                        )
                        # h_t = a_t * h_{t-1} + b_t along seq
                        h_t = h_big[m][:, s5 * 512:(s5 + 1) * 512]
                        init = 0.0 if last_h[m] is None else last_h[m][:, -1:]
                        nc.vector.tensor_tensor_scan(
                            h_t, a_t[:], b_t[:], init,
                            op0=OP.mult, op1=OP.add,
                        )
                        last_h[m] = h_t
                # Final chunk's outputs go via HWDGE (sync) -- the SWDGE
                # path adds a slow GpSimd drain right at the kernel tail.
                out_eng = nc.sync if c == NCHUNK - 1 else nc.gpsimd
                for mm in range(NM):
                    out_eng.dma_start(
                        out=hT[mm * 128:(mm + 1) * 128,
                               c * CHUNK:(c + 1) * CHUNK],
                        in_=h_big[mm][:],
                    )
    _split_sync_waits(nc)
    return nc


_NC_CACHE = None


def _get_program():
    global _NC_CACHE
    if _NC_CACHE is None:
        _NC_CACHE = _build_program()
    return _NC_CACHE


def _make_in_maps(x, Wz, bz, Wh, bh):
    xT16 = [np.ascontiguousarray(x[b].T).astype(np.float16) for b in range(BATCH)]
    wzh = [np.ascontiguousarray(Wz[:, c * DH:(c + 1) * DH]).astype(np.float16)
           for c in range(2)]
    whh = [np.ascontiguousarray(Wh[:, c * DH:(c + 1) * DH]).astype(np.float16)
           for c in range(2)]
    # bias[p, m] = bz[m*128+p]; columns [0:NM]=bz, [NM:2NM]=-bz, [2NM:3NM]=bh
    biases = []
    for c in range(2):
        bzc = bz[c * DH:(c + 1) * DH].astype(np.float32).reshape(NM, 128).T
        bhc = bh[c * DH:(c + 1) * DH].astype(np.float32).reshape(NM, 128).T
        biases.append(np.ascontiguousarray(np.hstack([bzc, -bzc, bhc])))
    in_maps = []
    for i in range(N_CORES):
        b, c = i // 2, i % 2
        in_maps.append({
            "xT": xT16[b], "wz": wzh[c], "wh": whh[c], "bias": biases[c],
        })
    return in_maps


def _run(x, Wz, bz, Wh, bh, trace=False, trace_cores=None):
    nc = _get_program()
    in_maps = _make_in_maps(x, Wz, bz, Wh, bh)
    res = run_bass_kernel_spmd(
        nc, in_maps, list(range(N_CORES)), trace=trace, trace_cores=trace_cores
    )
    out = np.empty((BATCH, SEQ, D), dtype=np.float32)
    for i in range(N_CORES):
        b, c = i // 2, i % 2
        out[b, :, c * DH:(c + 1) * DH] = res.results[i]["hT"].T
    return out, res


def kernel(x, Wz, bz, Wh, bh):
    x = np.asarray(x, dtype=np.float32)
    Wz = np.asarray(Wz, dtype=np.float32)
    Wh = np.asarray(Wh, dtype=np.float32)
    bz = np.asarray(bz, dtype=np.float32)
    bh = np.asarray(bh, dtype=np.float32)
    out, _ = _run(x, Wz, bz, Wh, bh, trace=False)
    return out
